# revision 9
# baseline (speedup 1.0000x reference)
"""Depthwise 3x3 blur of |x| on 8 trn2 NeuronCores (pure data-parallel on batch).

out[n,c] = corr2d(|x[n,c]|, w3x3, pad=1)  with w3x3 = weight[c,0] (same for all c).

Per-core plan (core i owns batch i: [16, 1024, 1024] f32):
  For each channel, 9 row-tiles. Each tile loads K<=128 consecutive image rows
  into SBUF (partition = image row), takes |.| on VectorE, then applies the
  conv as 3 column-shifted banded matmuls on TensorE accumulating in PSUM:
  matmul j applies kernel column j vertically via a banded lhsT[k, m] =
  w3x3[k-m(+1), j], while the +-1 horizontal shift comes from offsetting the
  rhs/out column ranges (out-of-range columns drop out => zero padding).
  PSUM (fp32) is evicted on ScalarE and DMA'd back.
"""

import numpy as np

import concourse.mybir as mybir
from concourse import bacc
from concourse.bass import MemorySpace
from concourse.bass_utils import run_bass_kernel_spmd
from concourse.tile import TileContext

N, C, H, W = 8, 16, 1024, 1024
P = 128  # SBUF partitions
MI = 126  # out rows per interior tile
BANK = 512  # fp32 elements per PSUM bank
F32 = mybir.dt.float32
F32R = mybir.dt.float32r


def _tile_plan():
    """Per-channel row tiling: (rhs_start, K, out_start, M, variant).

    variant 0 = interior bands  B[k, m] = w[k - m, j]   (rhs starts 1 row above out)
    variant 1 = first-row bands Cb[k, m] = w[k - m + 1, j] (rhs starts at out row 0)
    """
    plan = [(0, 127, 0, 126, 1)]
    for t in range(1, 8):
        plan.append((126 * t - 1, 128, 126 * t, 126, 0))
    plan.append((1007, 17, 1008, 16, 0))
    assert sum(m for _, _, _, m, _ in plan) == H
    return plan


def _build_bands(w3x3: np.ndarray) -> np.ndarray:
    """[6, 128, 128] f32: j=0..2 interior bands, 3..5 first-tile bands."""
    bands = np.zeros((6, P, P), np.float32)
    for j in range(3):
        for d in range(3):  # d = k - m (interior) / k - m + 1 (first)
            for m in range(MI):
                k = m + d
                if k < P:
                    bands[j, k, m] = w3x3[d, j]
                k = m + d - 1
                if 0 <= k < P:
                    bands[3 + j, k, m] = w3x3[d, j]
    return bands


def _gen_program():
    nc = bacc.Bacc("TRN2", target_bir_lowering=False, debug=False, num_devices=N)

    x = nc.dram_tensor("x", [C, H, W], F32, kind="ExternalInput")
    bands = nc.dram_tensor("bands", [6, P, P], F32R, kind="ExternalInput")
    zpad = nc.dram_tensor("zpad", [P, 2], F32R, kind="ExternalInput")
    out = nc.dram_tensor("out", [C, H, W], F32, kind="ExternalOutput")

    plan = _tile_plan()

    with TileContext(nc) as tc:
        with (
            tc.tile_pool(name="consts", bufs=1) as cpool,
            tc.tile_pool(name="xin", bufs=4) as xpool,
            tc.tile_pool(name="xabs", bufs=4) as apool,
            tc.tile_pool(name="oev", bufs=4) as opool,
            tc.tile_pool(name="ps", bufs=3, space=MemorySpace.PSUM) as pspool,
        ):
            bt = cpool.tile([P, 6 * P], F32R)
            for j in range(6):
                nc.sync.dma_start(out=bt[:, P * j : P * (j + 1)], in_=bands[j])
            btr = bt[:]

            for c in range(C):
                for rhs_start, K, out_start, M, variant in plan:
                    kload = min(P, H - rhs_start)  # keep DMAs at 128 partitions
                    xt = xpool.tile([P, W], F32)
                    nc.sync.dma_start(
                        out=xt[:kload], in_=x[c, rhs_start : rhs_start + kload]
                    )
                    # |x| tile with one zero column of padding on each side
                    # (cols 1..W hold data); pads let every shifted matmul be
                    # a full even-aligned 512-wide op (f32r requirement).
                    at = apool.tile([P, W + 2], F32R)
                    nc.sync.dma_start(out=at[:K, 0 : W + 2 : W + 1], in_=zpad[:K])
                    nc.scalar.activation(
                        at[:K, 1 : W + 1], xt[:K], mybir.ActivationFunctionType.Abs
                    )

                    ps = pspool.tile([P, W], F32)
                    for b in range(2):
                        c0 = BANK * b
                        lhs = lambda j: btr[:K, P * (3 * variant + j) : P * (3 * variant + j) + M]
                        # out col cc takes rhs data col cc+dj-1 = at col cc+dj
                        nc.tensor.matmul(
                            ps[:M, c0 : c0 + BANK],
                            lhs(1),
                            at[:K, c0 + 1 : c0 + 1 + BANK],
                            start=True,
                            stop=False,
                        )
                        nc.tensor.matmul(
                            ps[:M, c0 : c0 + BANK],
                            lhs(0),
                            at[:K, c0 : c0 + BANK],
                            start=False,
                            stop=False,
                        )
                        nc.tensor.matmul(
                            ps[:M, c0 : c0 + BANK],
                            lhs(2),
                            at[:K, c0 + 2 : c0 + 2 + BANK],
                            start=False,
                            stop=True,
                        )

                    ot = opool.tile([P, W], F32)
                    nc.vector.tensor_copy(out=ot[:M], in_=ps[:M])
                    nc.sync.dma_start(
                        out=out[c, out_start : out_start + M], in_=ot[:M]
                    )

    nc.compile()
    return nc


def _run(x: np.ndarray, weight: np.ndarray, trace: bool = False, tmpdir=None):
    assert x.shape == (N, C, H, W), x.shape
    w3x3 = np.asarray(weight, np.float32)[0, 0]
    bands = _build_bands(w3x3)

    nc = _gen_program()
    zpad = np.zeros((P, 2), np.float32)
    in_maps = [
        {"x": np.ascontiguousarray(x[i], np.float32), "bands": bands, "zpad": zpad}
        for i in range(N)
    ]
    res = run_bass_kernel_spmd(
        nc, in_maps, core_ids=list(range(N)), trace=trace, tmpdir=tmpdir
    )
    out = np.stack([res.results[i]["out"] for i in range(N)])
    return out, res


def kernel(x: np.ndarray, weight: np.ndarray) -> np.ndarray:
    out, _ = _run(np.asarray(x), np.asarray(weight))
    return out


# revision 10
# speedup vs baseline: 1.1121x; 1.1121x over previous
"""Depthwise 3x3 blur of |x| on 8 trn2 NeuronCores (pure data-parallel on batch).

out[n,c] = corr2d(|x[n,c]|, w3x3, pad=1)  with w3x3 = weight[c,0] (same for all c).

Per-core plan (core i owns batch i: [16, 1024, 1024] f32):
  x is host-padded with one zero column on each side (W+2 = 1026). For each
  channel, 9 row-tiles. Each tile loads K<=128 consecutive padded image rows
  into SBUF (partition = image row), takes |.| on ScalarE (casting to the
  matmul dtype), then applies the conv as 3 column-shifted banded matmuls on
  TensorE accumulating in PSUM: matmul j applies kernel column j vertically
  via a banded lhsT[k, m] = w3x3[k-m(+1), j], while the +-1 horizontal shift
  comes from offsetting the rhs column window over the padded tile (the zero
  pad columns supply the horizontal zero padding). PSUM (fp32) is evicted on
  VectorE and DMA'd back.
"""

import numpy as np

import concourse.mybir as mybir
from concourse import bacc
from concourse.bass import MemorySpace
from concourse.bass_utils import run_bass_kernel_spmd
from concourse.tile import TileContext

N, C, H, W = 8, 16, 1024, 1024
P = 128  # SBUF partitions
MI = 126  # out rows per interior tile
BANK = 512  # fp32 elements per PSUM bank
F32 = mybir.dt.float32

DTYPE = "bf16"  # matmul operand dtype: "bf16" or "f32r"


def _mm_dt():
    return mybir.dt.bfloat16 if DTYPE == "bf16" else mybir.dt.float32r


def _tile_plan():
    """Per-channel row tiling: (rhs_start, K, out_start, M, variant).

    variant 0 = interior bands  B[k, m] = w[k - m, j]   (rhs starts 1 row above out)
    variant 1 = first-row bands Cb[k, m] = w[k - m + 1, j] (rhs starts at out row 0)
    """
    plan = [(0, 127, 0, 126, 1)]
    for t in range(1, 8):
        plan.append((126 * t - 1, 128, 126 * t, 126, 0))
    plan.append((1007, 17, 1008, 16, 0))
    assert sum(m for _, _, _, m, _ in plan) == H
    return plan


def _build_bands(w3x3: np.ndarray) -> np.ndarray:
    """[6, 128, 128] f32: j=0..2 interior bands, 3..5 first-tile bands."""
    bands = np.zeros((6, P, P), np.float32)
    for j in range(3):
        for d in range(3):  # d = k - m (interior) / k - m + 1 (first)
            for m in range(MI):
                k = m + d
                if k < P:
                    bands[j, k, m] = w3x3[d, j]
                k = m + d - 1
                if 0 <= k < P:
                    bands[3 + j, k, m] = w3x3[d, j]
    return bands


def _gen_program():
    mmdt = _mm_dt()
    nc = bacc.Bacc("TRN2", target_bir_lowering=False, debug=False, num_devices=N)

    x = nc.dram_tensor("x", [C, H, W + 2], F32, kind="ExternalInput")
    bands = nc.dram_tensor("bands", [6, P, P], mmdt, kind="ExternalInput")
    out = nc.dram_tensor("out", [C, H, W], F32, kind="ExternalOutput")

    plan = _tile_plan()

    with TileContext(nc) as tc:
        with (
            tc.tile_pool(name="consts", bufs=1) as cpool,
            tc.tile_pool(name="xin", bufs=4) as xpool,
            tc.tile_pool(name="xabs", bufs=4) as apool,
            tc.tile_pool(name="oev", bufs=4) as opool,
            tc.tile_pool(name="ps", bufs=3, space=MemorySpace.PSUM) as pspool,
        ):
            bt = cpool.tile([P, 6 * P], mmdt)
            for j in range(6):
                nc.sync.dma_start(out=bt[:, P * j : P * (j + 1)], in_=bands[j])

            for c in range(C):
                for rhs_start, K, out_start, M, variant in plan:
                    kload = min(P, H - rhs_start)  # keep DMAs at 128 partitions
                    xt = xpool.tile([P, W + 2], F32)
                    nc.sync.dma_start(
                        out=xt[:kload], in_=x[c, rhs_start : rhs_start + kload]
                    )
                    # |x| (+ cast); pad columns stay zero: |0| = 0
                    at = apool.tile([P, W + 2], mmdt)
                    nc.scalar.activation(
                        at[:K], xt[:K], mybir.ActivationFunctionType.Abs
                    )

                    ps = pspool.tile([P, W], F32)
                    for b in range(2):
                        c0 = BANK * b
                        lhs = lambda j: bt[:K, P * (3 * variant + j) : P * (3 * variant + j) + P]
                        # out col cc takes rhs data col cc+dj-1 = padded col cc+dj
                        for i, j in enumerate((1, 0, 2)):
                            nc.tensor.matmul(
                                ps[:, c0 : c0 + BANK],
                                lhs(j),
                                at[:K, c0 + j : c0 + j + BANK],
                                start=(i == 0),
                                stop=(i == 2),
                            )

                    ot = opool.tile([P, W], F32)
                    nc.vector.tensor_copy(out=ot[:M], in_=ps[:M])
                    nc.sync.dma_start(
                        out=out[c, out_start : out_start + M], in_=ot[:M]
                    )

    nc.compile()
    return nc


def _run(x: np.ndarray, weight: np.ndarray, trace: bool = False, tmpdir=None):
    assert x.shape == (N, C, H, W), x.shape
    w3x3 = np.asarray(weight, np.float32)[0, 0]
    np_mmdt = mybir.dt.np(_mm_dt())
    bands = _build_bands(w3x3).astype(np_mmdt)

    xp = np.pad(np.asarray(x, np.float32), ((0, 0), (0, 0), (0, 0), (1, 1)))

    nc = _gen_program()
    in_maps = [
        {"x": np.ascontiguousarray(xp[i]), "bands": bands} for i in range(N)
    ]
    res = run_bass_kernel_spmd(
        nc, in_maps, core_ids=list(range(N)), trace=trace, tmpdir=tmpdir
    )
    out = np.stack([res.results[i]["out"] for i in range(N)])
    return out, res


def kernel(x: np.ndarray, weight: np.ndarray) -> np.ndarray:
    out, _ = _run(np.asarray(x), np.asarray(weight))
    return out


# revision 11
# speedup vs baseline: 1.6073x; 1.4453x over previous
"""Depthwise 3x3 blur of |x| on 8 trn2 NeuronCores (pure data-parallel on batch).

out[n,c] = corr2d(|x[n,c]|, w3x3, pad=1)  with w3x3 = weight[c,0] (same for all c).

Per-core plan (core i owns batch i: [16, 1024, 1024] f32):
  x is host-padded with one zero row/column on every side -> [C, 1026, 1026].
  Each channel is processed as 9 row-tiles: 8 tiles of 126 output rows plus a
  16-row tail. A tile's 128 padded input rows land in SBUF partitions
  (partition = image row), |.| runs on ScalarE (casting to the matmul dtype),
  and the conv is 3 column-shifted banded matmuls per 512-wide PSUM bank on
  TensorE: matmul j applies kernel column j vertically via a banded
  lhsT[k, m] = w3x3[k-m, j], while the +-1 horizontal shift comes from
  offsetting the rhs column window over the padded tile (pad columns supply
  the horizontal zero padding, pad rows the vertical). PSUM (fp32) is evicted
  on ScalarE/VectorE and DMA'd back.

  DMA: 4 row-tiles are loaded per dma_start (2 MiB, overlapping 128-row
  chunks at stride 126 via a raw access pattern) on the Sync HWDGE queue, and
  4 output tiles are stored per dma_start (2 MiB) on the GpSimd SWDGE queue,
  so loads and stores run on independent DMA queues.
"""

import numpy as np

import concourse.mybir as mybir
from concourse.ap import AP
from concourse import bacc
from concourse.bass import MemorySpace
from concourse.bass_utils import run_bass_kernel_spmd
from concourse.tile import TileContext

N, C, H, W = 8, 16, 1024, 1024
P = 128  # SBUF partitions
MI = 126  # out rows per regular tile
BANK = 512  # fp32 elements per PSUM bank
HP, WP = H + 2, W + 2  # padded image dims
F32 = mybir.dt.float32

DTYPE = "bf16"  # matmul operand dtype: "bf16" or "f32r"


def _mm_dt():
    return mybir.dt.bfloat16 if DTYPE == "bf16" else mybir.dt.float32r


def _build_bands(w3x3: np.ndarray) -> np.ndarray:
    """[3, 128, 128] f32 banded lhsT: B[j][k, m] = w3x3[k - m, j]."""
    bands = np.zeros((3, P, P), np.float32)
    for j in range(3):
        for d in range(3):
            for m in range(MI):
                if m + d < P:
                    bands[j, m + d, m] = w3x3[d, j]
    return bands


def _matmuls(nc, ps, bt, at, at_col0, K):
    """3 column-shifted banded matmuls per 512-wide PSUM bank of ps."""
    nbank = ps.shape[1] // BANK
    for b in range(nbank):
        c0 = BANK * b
        for i, j in enumerate((1, 0, 2)):
            nc.tensor.matmul(
                ps[:, c0 : c0 + BANK],
                bt[:K, P * j : P * (j + 1)],
                at[:K, at_col0 + c0 + j : at_col0 + c0 + j + BANK],
                start=(i == 0),
                stop=(i == 2),
            )


def _gen_program():
    mmdt = _mm_dt()
    nc = bacc.Bacc("TRN2", target_bir_lowering=False, debug=False, num_devices=N)

    x = nc.dram_tensor("x", [C, HP, WP], F32, kind="ExternalInput")
    bands = nc.dram_tensor("bands", [3, P, P], mmdt, kind="ExternalInput")
    out = nc.dram_tensor("out", [C, H, W], F32, kind="ExternalOutput")

    with TileContext(nc) as tc:
        with (
            tc.tile_pool(name="consts", bufs=1) as cpool,
            tc.tile_pool(name="xin", bufs=3) as xpool,
            tc.tile_pool(name="xabs", bufs=3) as apool,
            tc.tile_pool(name="oev", bufs=3) as opool,
            tc.tile_pool(name="ps", bufs=3, space=MemorySpace.PSUM) as pspool,
        ):
            bt = cpool.tile([P, 3 * P], mmdt)
            for j in range(3):
                nc.sync.dma_start(out=bt[:, P * j : P * (j + 1)], in_=bands[j])

            for c in range(C):
                for q in range(2):  # quads of 4 row-tiles: t = 4q + k
                    r0 = 504 * q  # padded row of chunk 0
                    xt = xpool.tile([P, 4 * WP], F32)
                    src = AP(
                        x, c * HP * WP + r0 * WP,
                        [[WP, P], [MI * WP, 4], [1, WP]],
                    )
                    nc.sync.dma_start(out=xt[:], in_=src)

                    at = apool.tile([P, 4 * WP], mmdt)
                    nc.scalar.activation(
                        at[:], xt[:], mybir.ActivationFunctionType.Abs
                    )

                    ot = opool.tile([P, 4 * W], F32)
                    for k in range(4):
                        ps = pspool.tile([P, W], F32)
                        _matmuls(nc, ps, bt, at, k * WP, P)
                        ev = nc.vector.tensor_copy if k % 2 else nc.scalar.copy
                        ev(ot[:MI, k * W : (k + 1) * W], ps[:MI])

                    dst = AP(
                        out, c * H * W + 4 * MI * q * W,
                        [[W, MI], [MI * W, 4], [1, W]],
                    )
                    nc.gpsimd.dma_start(out=dst, in_=ot[:MI, :])

                # tail: out rows 1008..1023 (M=16), padded rhs rows 1008..1025
                K8, M8 = 18, 16
                xt = xpool.tile([P, 4 * WP], F32)
                nc.sync.dma_start(out=xt[:K8, :WP], in_=x[c, 1008 : 1008 + K8])
                at = apool.tile([P, 4 * WP], mmdt)
                nc.scalar.activation(
                    at[:K8, :WP], xt[:K8, :WP], mybir.ActivationFunctionType.Abs
                )
                ps = pspool.tile([P, W], F32)
                _matmuls(nc, ps, bt, at, 0, K8)
                ot = opool.tile([P, 4 * W], F32)
                nc.vector.tensor_copy(ot[:M8, :W], ps[:M8])
                nc.gpsimd.dma_start(out=out[c, 8 * MI :], in_=ot[:M8, :W])

    nc.compile()
    return nc


def _run(x: np.ndarray, weight: np.ndarray, trace: bool = False, tmpdir=None):
    assert x.shape == (N, C, H, W), x.shape
    w3x3 = np.asarray(weight, np.float32)[0, 0]
    np_mmdt = mybir.dt.np(_mm_dt())
    bands = _build_bands(w3x3).astype(np_mmdt)

    xp = np.pad(np.asarray(x, np.float32), ((0, 0), (0, 0), (1, 1), (1, 1)))

    nc = _gen_program()
    in_maps = [
        {"x": np.ascontiguousarray(xp[i]), "bands": bands} for i in range(N)
    ]
    res = run_bass_kernel_spmd(
        nc, in_maps, core_ids=list(range(N)), trace=trace, tmpdir=tmpdir
    )
    out = np.stack([res.results[i]["out"] for i in range(N)])
    return out, res


def kernel(x: np.ndarray, weight: np.ndarray) -> np.ndarray:
    out, _ = _run(np.asarray(x), np.asarray(weight))
    return out
